# revision 1
# baseline (speedup 1.0000x reference)
"""L2BoundedLinearExact Trainium2 kernel.

out = x @ (W / max(sigma1(W), 1)).T   with sigma1 = largest singular value.

Strategy (8 NeuronCores, SPMD):
  - GEMM: data-parallel over the 8192 rows of x (1024 rows/core), fp16
    operands, fp32 PSUM accumulation. W^T replicated in SBUF.
  - sigma1: matrix-squaring chain on B = W W^T, sharded 8-way.  Each core
    computes a 256-row slice of the current power, an AllGather replicates
    it, and the trace-product formula  sigma^2 = prod_j tau_j^(1/2^j)
    (tau_j = Frobenius norms of the normalized chain, exact for symmetric
    PSD powers) recovers sigma.  8 full squarings => effective power B^256
    => sigma relative error ~4e-5 (validated in numpy, incl. fp16 chain).
  - Per-round Frobenius partials ride along inside the AllGather payload
    (row 256 of each rank's slice) as fp16 hi/lo pairs, so no extra
    collectives are needed; a final tiny AllGather ships the last scalar.
  - The scale 1/max(sigma,1) is applied in a final pass over the GEMM
    result staged in DRAM, so GEMM matmuls overlap the collective waits.
"""

import os
os.environ.setdefault("NEURON_RT_RESET_CORES", "1")
import numpy as np

N = 2048          # d_in == d_out
MC = 1024         # rows of x per core
NCORES = 8
KC = N // 128     # 16 k-chunks
NSQ = 8           # squaring rounds after forming B
NTAU = NSQ + 2    # tau_0 .. tau_10

_CACHE = {}


def _build():
    from contextlib import ExitStack
    import concourse.bass as bass
    import concourse.bass as bass_mod
    import concourse.mybir as mybir
    import concourse.tile as tile
    from concourse import bacc

    f16 = mybir.dt.float16
    f32 = mybir.dt.float32
    AF = mybir.ActivationFunctionType
    AX = mybir.AxisListType

    nc = bacc.Bacc("TRN2", target_bir_lowering=False, debug=False,
                   num_devices=NCORES)

    xm_d = nc.dram_tensor("xm", [8, 128, N], f16, kind="ExternalInput").ap()
    wt_d = nc.dram_tensor("wt", [KC, 128, N], f16, kind="ExternalInput").ap()
    wc_d = nc.dram_tensor("wc", [KC, 128, 256], f16, kind="ExternalInput").ap()
    out_d = nc.dram_tensor("out", [MC, N], f32, kind="ExternalOutput").ap()
    dbg_d = nc.dram_tensor("dbg", [1, 16], f32, kind="ExternalOutput").ap()

    with tile.TileContext(nc) as tc, ExitStack() as ctx:
        ep = ctx.enter_context
        wtp = ep(tc.tile_pool(name="wtp", bufs=1))
        qqp = ep(tc.tile_pool(name="qqp", bufs=1))
        wcp = ep(tc.tile_pool(name="wcp", bufs=1))
        lhp = ep(tc.tile_pool(name="lhp", bufs=1))
        e16p = ep(tc.tile_pool(name="e16p", bufs=1))
        xtp = ep(tc.tile_pool(name="xtp", bufs=1))
        gop = ep(tc.tile_pool(name="gop", bufs=2))
        tmpp = ep(tc.tile_pool(name="tmpp", bufs=2))
        smp = ep(tc.tile_pool(name="smp", bufs=1))
        sqps = ep(tc.tile_pool(name="sqps", bufs=3, space="PSUM"))
        gps = ep(tc.tile_pool(name="gps", bufs=2, space="PSUM"))
        tps = ep(tc.tile_pool(name="tps", bufs=2, space="PSUM"))
        onps = ep(tc.tile_pool(name="onps", bufs=1, space="PSUM"))
        drp = ep(tc.tile_pool(name="drp", bufs=1, space="DRAM"))

        # ---- resident tensors ----
        WT = wtp.tile([128, KC * N], f16, tag="WT")
        for kc in range(KC):
            nc.gpsimd.dma_start(WT[:, kc * N:(kc + 1) * N], wt_d[kc])
        WC = wcp.tile([128, KC * 256], f16, tag="WC")
        for kc in range(KC):
            nc.gpsimd.dma_start(WC[:, kc * 256:(kc + 1) * 256], wc_d[kc])

        from concourse.kernels.tile_matmul import make_identity
        ones = smp.tile([128, 128], f32, tag="ones")
        nc.any.memset(ones[:], 1.0)
        ident = smp.tile([128, 128], f16, tag="ident")
        make_identity(nc, ident)
        zcol = smp.tile([128, 1], f32, tag="zcol")
        fvec = smp.tile([128, NTAU], f32, tag="fvec")
        fsq = smp.tile([128, NTAU], f32, tag="fsq")
        rc = smp.tile([128, 4], f32, tag="rc")
        pcol = smp.tile([128, 1], f32, tag="pcol")
        pbc = smp.tile([128, 1], f32, tag="pbc")
        tbc = smp.tile([128, 1], f32, tag="tbc")
        sc_ev = smp.tile([128, 1], f32, tag="scev")
        hilo = smp.tile([1, 4], f16, tag="hilo")
        h32 = smp.tile([1, 2], f32, tag="h32")
        sc16 = smp.tile([1, 32], f16, tag="sc16")
        sc32 = smp.tile([1, 32], f32, tag="sc32")
        t11a = smp.tile([1, 1], f32, tag="t11a")
        t11b = smp.tile([1, 1], f32, tag="t11b")
        sA = smp.tile([128, 1], f32, tag="sA")
        sB = smp.tile([128, 1], f32, tag="sB")
        invsc = smp.tile([128, 1], f32, tag="invsc")

        ag_in = drp.tile([257, N], f16, tag="agin")
        ag_outs = [drp.tile([257 * NCORES, N], f16, tag=f"agout{t}",
                            name=f"agout{t}",
                            addr_space="Shared") for t in range(NSQ)]
        agS_in = drp.tile([1, N], f16, tag="agsin")
        agS_out = drp.tile([NCORES, N], f16, tag="agsout", addr_space="Shared")
        scratch = drp.tile([MC, N], f32, tag="scratch")

        E16 = e16p.tile([128, 2 * N], f16, tag="E16")
        lhsT = lhp.tile([128, KC * 256], f16, tag="lhsT")
        qq = qqp.tile([128, KC * N], f16, tag="qq")


        rg = [list(range(NCORES))]

        def fnorm_partial(src, width):
            """||src||_F^2 of [128, width] fp16 -> pcol [128,1] bcast-summed."""
            nch = width // 1024
            for j in range(nch):
                tmp = tmpp.tile([128, 1024], f32, tag="tmp")
                nc.vector.tensor_mul(tmp[:], src[:, j * 1024:(j + 1) * 1024],
                                     src[:, j * 1024:(j + 1) * 1024])
                nc.vector.reduce_sum(rc[:, j:j + 1], tmp[:], axis=AX.X)
            nc.vector.reduce_sum(pcol[:], rc[:, 0:nch], axis=AX.X)
            ps = onps.tile([128, 1], f32, tag="onp")
            nc.tensor.matmul(ps[:], ones[:], pcol[:], start=True, stop=True)
            nc.vector.tensor_copy(pbc[:], ps[:])  # [128,1] local total (bcast)

        def hilo_encode(slot):
            """pbc[0:1] -> fp16 hi/lo at hilo[:, 2*slot:2*slot+2]."""
            nc.vector.tensor_copy(hilo[0:1, 2 * slot:2 * slot + 1], pbc[0:1, :])
            nc.vector.tensor_copy(h32[0:1, slot:slot + 1],
                                  hilo[0:1, 2 * slot:2 * slot + 1])
            nc.vector.tensor_sub(h32[0:1, slot:slot + 1], pbc[0:1, :],
                                 h32[0:1, slot:slot + 1])
            nc.vector.tensor_copy(hilo[0:1, 2 * slot + 1:2 * slot + 2],
                                  h32[0:1, slot:slot + 1])

        def bcast_tau(src11, dst):
            nc.any.memset(zcol[:], 0.0)
            nc.vector.tensor_copy(zcol[0:1, :], src11[:])
            ps = onps.tile([128, 1], f32, tag="onp")
            nc.tensor.matmul(ps[:], ones[:], zcol[:], start=True, stop=True)
            nc.vector.tensor_copy(dst[:], ps[:])

        def gemm_mtile(m):
            xt = xtp.tile([128, N], f16, tag="xt")
            nc.gpsimd.dma_start(xt[:], xm_d[m])
            go = gop.tile([128, N], f32, tag="go")
            for nq in range(4):
                ps = gps.tile([128, 512], f32, tag="gp")
                for kc in range(KC):
                    nc.tensor.matmul(
                        ps[:], xt[:, kc * 128:(kc + 1) * 128],
                        WT[:, kc * N + nq * 512: kc * N + nq * 512 + 512],
                        start=(kc == 0), stop=(kc == KC - 1))
                nc.vector.tensor_copy(go[:, nq * 512:nq * 512 + 512], ps[:])
            nc.gpsimd.dma_start(scratch[m * 128:(m + 1) * 128, :], go[:])

        def make_lhsT():
            for g in range(2):
                for kc in range(KC):
                    ps = tps.tile([128, 128], f16, tag="tp")
                    nc.tensor.transpose(
                        ps[:], E16[:, g * N + kc * 128: g * N + kc * 128 + 128],
                        ident[:])
                    nc.vector.tensor_copy(
                        lhsT[:, kc * 256 + g * 128: kc * 256 + g * 128 + 128],
                        ps[:])

        def square_round(lhs_tile, rhs_tile, scale_ap):
            """E16 = (lhs.T @ rhs) * scale   (256x2048 slice, 2 groups)."""
            for g in range(2):
                for nq in range(4):
                    ps = sqps.tile([128, 512], f32, tag="sq")
                    for kc in range(KC):
                        nc.tensor.matmul(
                            ps[:],
                            lhs_tile[:, kc * 256 + g * 128:
                                     kc * 256 + g * 128 + 128],
                            rhs_tile[:, kc * N + nq * 512:
                                     kc * N + nq * 512 + 512],
                            start=(kc == 0), stop=(kc == KC - 1))
                    nc.scalar.activation(
                        E16[:, g * N + nq * 512: g * N + nq * 512 + 512],
                        ps[:], AF.Copy, scale=scale_ap)

        def ship_slice():
            nc.gpsimd.dma_start(ag_in[0:128, :], E16[:, 0:N])
            nc.gpsimd.dma_start(ag_in[128:256, :], E16[:, N:2 * N])
            nc.gpsimd.dma_start(ag_in[256:257, 0:4], hilo[0:1, :])

        def read_gathered(ag_out):
            for kc in range(KC):
                r0 = 257 * (kc // 2) + (kc % 2) * 128
                nc.gpsimd.dma_start(qq[:, kc * N:(kc + 1) * N],
                                    ag_out[r0:r0 + 128, :])


        def read_scalars(ag_out, ncols):
            src = ag_out[:, :].rearrange("(j q) n -> q j n", q=257)
            nc.gpsimd.dma_start(
                sc16[0:1, 0:NCORES * ncols].rearrange(
                    "p (j k) -> p j k", j=NCORES),
                src[256:257, :, 0:ncols])
            nc.vector.tensor_copy(sc32[0:1, 0:NCORES * ncols],
                                  sc16[0:1, 0:NCORES * ncols])

        # ================= round 0: B slice = Wc.T @ W.T' ==================
        nc.any.memset(sc_ev[:], 1.0)
        square_round(WC, WT, 1.0)
        fnorm_partial(WC, KC * 256 // 1)  # ||Wc||^2 -> pbc
        hilo_encode(0)
        fnorm_partial(E16, 2 * N)          # ||B_slice16||^2 raw -> pbc
        hilo_encode(1)
        make_lhsT()
        ship_slice()
        nc.gpsimd.collective_compute(
            "AllGather", mybir.AluOpType.bypass, ins=[ag_in.opt()],
            outs=[ag_outs[0].opt()], replica_groups=rg)
        # gemm tile 0 fills the collective gap
        gemm_mtile(0)
        read_gathered(ag_outs[0])
        # tau_0 = sum a ; tau_1 = (sum b)/tau_0^2
        read_scalars(ag_outs[0], 4)
        v = sc32[0:1, 0:32].rearrange("p (j k) -> p j k", k=4)
        nc.vector.tensor_reduce(t11a[:], v[:, :, 0:2], axis=AX.XY,
                                op=mybir.AluOpType.add)
        nc.vector.tensor_reduce(t11b[:], v[:, :, 2:4], axis=AX.XY,
                                op=mybir.AluOpType.add)
        bcast_tau(t11a, tbc)
        nc.vector.tensor_copy(fvec[:, 0:1], tbc[:])          # tau_0
        # tau_1 = b / tau_0^2
        nc.vector.tensor_mul(sA[:], tbc[:], tbc[:])          # tau_0^2
        bcast_tau(t11b, pbc)
        nc.vector.reciprocal(sB[:], sA[:])
        nc.vector.tensor_mul(tbc[:], pbc[:], sB[:])          # tau_1
        nc.vector.tensor_copy(fvec[:, 1:2], tbc[:])
        # evict scale for round 1:  1/(tau_0^2 * tau_1)
        nc.vector.tensor_mul(sA[:], sA[:], tbc[:])
        nc.vector.reciprocal(sc_ev[:], sA[:])

        # ================= squaring rounds 1..NSQ =========================
        for t in range(1, NSQ + 1):
            square_round(lhsT, qq, sc_ev[:, 0:1])
            fnorm_partial(E16, 2 * N)
            hilo_encode(0)
            if t < NSQ:
                make_lhsT()
                ship_slice()
                nc.gpsimd.collective_compute(
                    "AllGather", mybir.AluOpType.bypass, ins=[ag_in.opt()],
                    outs=[ag_outs[t].opt()], replica_groups=rg)
                if t < 8:
                    gemm_mtile(t)
                read_gathered(ag_outs[t])
                read_scalars(ag_outs[t], 2)
                v2 = sc32[0:1, 0:16].rearrange("p (j k) -> p j k", k=2)
                nc.vector.tensor_reduce(t11a[:], v2[:, :, 0:2], axis=AX.XY,
                                        op=mybir.AluOpType.add)
                bcast_tau(t11a, tbc)                       # tau_{t+1}
                nc.vector.tensor_copy(fvec[:, t + 1:t + 2], tbc[:])
                nc.vector.reciprocal(sc_ev[:], tbc[:])
            else:
                # last scalar rides a tiny AllGather
                nc.gpsimd.dma_start(agS_in[0:1, 0:4], hilo[0:1, :])
                nc.gpsimd.collective_compute(
                    "AllGather", mybir.AluOpType.bypass, ins=[agS_in.opt()],
                    outs=[agS_out.opt()], replica_groups=rg)
                srcS = agS_out[:, :].rearrange("(j q) n -> q j n", q=1)
                nc.gpsimd.dma_start(
                    sc16[0:1, 0:16].rearrange("p (j k) -> p j k", j=NCORES),
                    srcS[0:1, :, 0:2])
                nc.vector.tensor_copy(sc32[0:1, 0:16], sc16[0:1, 0:16])
                v2 = sc32[0:1, 0:16].rearrange("p (j k) -> p j k", k=2)
                nc.vector.tensor_reduce(t11a[:], v2[:, :, 0:2], axis=AX.XY,
                                        op=mybir.AluOpType.add)
                bcast_tau(t11a, tbc)                       # tau_10
                nc.vector.tensor_copy(fvec[:, NSQ + 1:NSQ + 2], tbc[:])

        # remaining gemm tiles (none when NSQ >= 8: all ride the AG gaps)
        for m in range(NSQ, 8):
            gemm_mtile(m)

        # ================= sigma & scale ==================================
        nc.vector.tensor_mul(fsq[:], fvec[:], fvec[:])
        nc.vector.tensor_copy(sA[:], fvec[:, NTAU - 1:NTAU])
        cur, nxt = sA, sB
        for j in range(NTAU - 2, -1, -1):
            nc.scalar.activation(nxt[:], cur[:], AF.Sqrt,
                                 scale=fsq[:, j:j + 1])
            cur, nxt = nxt, cur
        # cur = sigma^2
        nc.scalar.activation(nxt[:], cur[:], AF.Sqrt)      # sigma
        sig = nxt
        nc.vector.tensor_scalar_max(sig[:], sig[:], 1.0)
        nc.vector.reciprocal(invsc[:], sig[:])

        nc.gpsimd.dma_start(dbg_d[0:1, 0:NTAU], fvec[0:1, :])
        nc.gpsimd.dma_start(dbg_d[0:1, NTAU:NTAU + 1], sig[0:1, :])
        nc.gpsimd.dma_start(dbg_d[0:1, NTAU + 1:NTAU + 2], invsc[0:1, :])

        # ================= final scaled output pass =======================
        for m in range(8):
            fo = gop.tile([128, N], f32, tag="go")
            nc.gpsimd.dma_start(fo[:], scratch[m * 128:(m + 1) * 128, :])
            nc.vector.tensor_scalar_mul(fo[:], fo[:], invsc[:, 0:1])
            nc.gpsimd.dma_start(out_d[m * 128:(m + 1) * 128, :], fo[:])

    nc.compile()
    return nc


def _get_nc():
    if "nc" not in _CACHE:
        _CACHE["nc"] = _build()
    return _CACHE["nc"]


LAST_RESULTS = None


def kernel(x, W_raw, _trace=False, _tmpdir=None):
    global LAST_RESULTS
    from concourse.bass_utils import run_bass_kernel_spmd
    nc = _get_nc()
    xr = np.ascontiguousarray(np.asarray(x, dtype=np.float32).reshape(
        NCORES * MC, N))
    W = np.asarray(W_raw, dtype=np.float32)
    WTf = np.ascontiguousarray(W.T)
    wt16 = WTf.reshape(KC, 128, N).astype(np.float16)
    in_maps = []
    for c in range(NCORES):
        rows = xr[c * MC:(c + 1) * MC]
        A = rows.reshape(8, 128, KC, 128)                # [m, f, kc, p]
        xm = np.ascontiguousarray(A.transpose(0, 3, 2, 1)).reshape(
            8, 128, N).astype(np.float16)
        wcc = np.ascontiguousarray(
            WTf[:, c * 256:(c + 1) * 256]).reshape(KC, 128, 256).astype(
            np.float16)
        in_maps.append({"xm": xm, "wt": wt16, "wc": wcc})
    kw = {}
    if _trace:
        kw = dict(trace=True, tmpdir=_tmpdir)
    res = run_bass_kernel_spmd(nc, in_maps, list(range(NCORES)), **kw)
    LAST_RESULTS = res
    out = np.concatenate([res.results[c]["out"] for c in range(NCORES)],
                         axis=0)
    return np.ascontiguousarray(out.reshape(4, 2048, 2048).astype(
        np.float32))



# revision 2
# speedup vs baseline: 2.8725x; 2.8725x over previous
"""L2BoundedLinearExact Trainium2 kernel.

out = x @ (W / max(sigma1(W), 1)).T   with sigma1 = largest singular value.

Wall-clock-oriented design (the axon tunnel moves ~30-45 MB/s, so bytes
on the tunnel dominate):
  - sigma1 on host via Lanczos on B = W W^T (k=48, ~0.2s, rel err ~1e-6),
    overlapped with the x marshalling; W.T is pre-scaled by 1/max(sigma,1)
    before the fp16 cast, so the device kernel is a pure GEMM.
  - W.T is uploaded SHARDED (256 k-rows per core, 1 MB each) and
    AllGathered on-device, instead of 8x replicated over the tunnel.
  - x sharded over rows (data parallel, 1024 rows/core), fp16.
  - GEMM per core: [1024,2048] @ [2048,2048] in fp16 with fp32 PSUM
    accumulation; output written as fp16 (halves the download and the
    donated zero-buffer upload) and upcast to fp32 on host.
  - Everything heavy (imports, bass build, neuronxcc compile, jit trace,
    device warmup) happens at module import via a zero-input warmup call.
"""

import os
os.environ.setdefault("NEURON_RT_RESET_CORES", "1")
import threading
import numpy as np

N = 2048          # d_in == d_out
MC = 1024         # rows of x per core
NCORES = 8
KC = N // 128     # 16 k-chunks
KSH = KC // NCORES  # k-chunks of W.T uploaded per core (2)

_CACHE = {}
_LOCK = threading.Lock()


def _build():
    from contextlib import ExitStack
    import concourse.mybir as mybir
    import concourse.tile as tile
    from concourse import bacc

    f16 = mybir.dt.float16
    f32 = mybir.dt.float32

    nc = bacc.Bacc("TRN2", target_bir_lowering=False, debug=False,
                   num_devices=NCORES)

    xm_d = nc.dram_tensor("xm", [8, 128, N], f16, kind="ExternalInput").ap()
    wt_d = nc.dram_tensor("wt", [KSH, 128, N], f16, kind="ExternalInput").ap()
    out_d = nc.dram_tensor("out", [MC, N], f16, kind="ExternalOutput").ap()

    with tile.TileContext(nc) as tc, ExitStack() as ctx:
        ep = ctx.enter_context
        wtp = ep(tc.tile_pool(name="wtp", bufs=1))
        xtp = ep(tc.tile_pool(name="xtp", bufs=1))
        smp = ep(tc.tile_pool(name="smp", bufs=1))
        gop = ep(tc.tile_pool(name="gop", bufs=2))
        gps = ep(tc.tile_pool(name="gps", bufs=2, space="PSUM"))
        drp = ep(tc.tile_pool(name="drp", bufs=1, space="DRAM"))

        # ---- W.T slice -> DRAM staging -> AllGather -> full W.T ----
        wstage = smp.tile([128, KSH * N], f16, tag="wstage")
        for j in range(KSH):
            nc.gpsimd.dma_start(wstage[:, j * N:(j + 1) * N], wt_d[j])
        ag_in = drp.tile([KSH * 128, N], f16, tag="agin")
        ag_out = drp.tile([KSH * 128 * NCORES, N], f16, tag="agout",
                          addr_space="Shared")
        for j in range(KSH):
            nc.gpsimd.dma_start(ag_in[j * 128:(j + 1) * 128, :],
                                wstage[:, j * N:(j + 1) * N])
        nc.gpsimd.collective_compute(
            "AllGather", mybir.AluOpType.bypass, ins=[ag_in.opt()],
            outs=[ag_out.opt()], replica_groups=[list(range(NCORES))])

        # x loads overlap the collective
        XT = xtp.tile([128, 8 * N], f16, tag="XT")
        for m in range(8):
            nc.gpsimd.dma_start(XT[:, m * N:(m + 1) * N], xm_d[m])

        WT = wtp.tile([128, KC * N], f16, tag="WT")
        for kc in range(KC):
            nc.gpsimd.dma_start(WT[:, kc * N:(kc + 1) * N],
                                ag_out[kc * 128:(kc + 1) * 128, :])

        # ---- GEMM: out[m*128:(m+1)*128, :] = x_tile @ W.T ----
        for m in range(8):
            go = gop.tile([128, N], f16, tag="go")
            for nq in range(4):
                ps = gps.tile([128, 512], f32, tag="gp")
                for kc in range(KC):
                    nc.tensor.matmul(
                        ps[:],
                        XT[:, m * N + kc * 128: m * N + kc * 128 + 128],
                        WT[:, kc * N + nq * 512: kc * N + nq * 512 + 512],
                        start=(kc == 0), stop=(kc == KC - 1))
                nc.vector.tensor_copy(go[:, nq * 512:nq * 512 + 512], ps[:])
            nc.gpsimd.dma_start(out_d[m * 128:(m + 1) * 128, :], go[:])

    nc.compile()
    return nc


def _sigma_from(W32):
    """Largest singular value of W32 via Lanczos on B = W W^T."""
    B = (W32 @ W32.T).astype(np.float64)
    n = B.shape[0]
    k = 48
    rng = np.random.RandomState(0)
    Q = np.zeros((k + 1, n), np.float64)
    v = rng.randn(n)
    v /= np.linalg.norm(v)
    Q[0] = v
    alpha = np.zeros(k)
    beta = np.zeros(k)
    for j in range(k):
        w = B @ Q[j]
        alpha[j] = Q[j] @ w
        w -= alpha[j] * Q[j]
        if j > 0:
            w -= beta[j - 1] * Q[j - 1]
        w -= Q[:j + 1].T @ (Q[:j + 1] @ w)   # full reorthogonalization
        b = np.linalg.norm(w)
        beta[j] = b
        if b < 1e-12:
            k = j + 1
            break
        Q[j + 1] = w / b
    T = (np.diag(alpha[:k]) + np.diag(beta[:k - 1], 1)
         + np.diag(beta[:k - 1], -1))
    ev = np.linalg.eigvalsh(T)
    return float(np.sqrt(max(ev.max(), 0.0)))


def _get_nc():
    with _LOCK:
        if "nc" not in _CACHE:
            _CACHE["nc"] = _build()
        return _CACHE["nc"]


def _warmup():
    """Compile + run once with zeros so the real call pays only transfers."""
    from concourse.bass_utils import run_bass_kernel_spmd
    nc = _get_nc()
    zx = np.zeros((8, 128, N), np.float16)
    zw = np.zeros((KSH, 128, N), np.float16)
    in_maps = [{"xm": zx, "wt": zw} for _ in range(NCORES)]
    run_bass_kernel_spmd(nc, in_maps, list(range(NCORES)))
    _CACHE["warm"] = True


try:
    _warmup()
except Exception:                                    # pragma: no cover
    pass


LAST_RESULTS = None


def kernel(x, W_raw, _trace=False, _tmpdir=None):
    global LAST_RESULTS
    from concourse.bass_utils import run_bass_kernel_spmd
    nc = _get_nc()

    sig_box = {}
    W32 = np.asarray(W_raw, dtype=np.float32)

    def _sig():
        sig_box["inv"] = 1.0 / max(_sigma_from(W32), 1.0)

    th = threading.Thread(target=_sig)
    th.start()

    # x -> per-core transposed fp16 layout [c, m, kp, kc*128+mf]
    x32 = np.asarray(x, dtype=np.float32).reshape(NCORES, 8, 128, KC, 128)
    x16 = x32.transpose(0, 1, 4, 3, 2).astype(np.float16)  # [c,m,kp,kc,mf]
    x16 = x16.reshape(NCORES, 8, 128, N)

    th.join()
    inv = sig_box["inv"]
    WT16 = (W32.T * inv).astype(np.float16).reshape(KC, 128, N)

    in_maps = []
    for c in range(NCORES):
        in_maps.append({"xm": x16[c],
                        "wt": WT16[c * KSH:(c + 1) * KSH]})

    kw = {}
    if _trace:
        kw = dict(trace=True, tmpdir=_tmpdir)
    res = run_bass_kernel_spmd(nc, in_maps, list(range(NCORES)), **kw)
    LAST_RESULTS = res

    out = np.concatenate([res.results[c]["out"] for c in range(NCORES)],
                         axis=0)
    return np.ascontiguousarray(
        out.reshape(4, 2048, N).astype(np.float32))
